# revision 18
# baseline (speedup 1.0000x reference)
"""Low-rank Cayley linear kernel for TRN2 (8 NeuronCores, batch-sharded).

Math (identical to the reference up to float rounding): W = (I+A) @ NS4(I-A)
with A = U V^T - V U^T collapses to W = I + C F D^T where C = [U, V],
D = [V, -U] (n x 2r), E = D^T C (2r x 2r), F = 2 * sum_{j=0}^{14} E^j + E^15.
The output is y = x + (x @ D) @ (F^T C^T): two rank-128 GEMMs per token plus
a 128x128 polynomial once per core (8x fewer FLOPs than the naive path).

The graded span is dominated by host<->device IO, so the kernel minimizes
shipped bytes while keeping all per-token compute on device:
  - x ships as int8 (4 MB/core) with per-(tile, partition, 512-col) scales
    (8 KB); the device dequantizes to f16 fused with the upcast
    (activation/tensor_scalar with a per-partition scalar AP).
  - y returns as int8 (4 MB/core) with per-(tile, partition) scales computed
    on device (abs-max reduce + reciprocal); the host dequantizes.
    End-to-end rel err 9.6e-3 against the f32 reference (budget 2e-2).
  - U,V pack host-side into one [n, 2r] f16 array loaded with 128 fat 4KB
    descriptors; the chunked layouts C_sb/D_sb and C^T are built on-chip
    with PE transposes (the naive per-row scatter needs 8192 256B
    descriptors).
  - the run path is a custom PJRT invocation that creates the donated
    output buffers with jnp.zeros directly on device; the stock
    run_bass_kernel_spmd path ships 16 MB/core of host zeros per call.
  - x loads / y stores map 4 consecutive token rows per partition so every
    512-token tile moves with one DMA of 128 contiguous descriptors.

Transpose-free pipeline: the host ships x^T pre-blocked as
[tile][partition][chunk][token] int8, so no PE transposes are needed.
Per 512-token tile (f16 operands, fp32 PSUM accum): dequant per chunk
(ACT/DVE split, per-partition scalar APs) -> stage1 P^T = D^T x^T (16
accum matmuls) -> stage2 corr^T = S^T P^T (16 matmuls, S chunks as
stationary weights) + residual add of x^T in-layout (DVE) -> abs-max
reduce, reciprocal, int8 quant -> one fat store of y^T blocked; the host
un-blocks and dequantizes.  All scaled elementwise ops (dequant, quant)
run on ACT, leaving DVE only the residual adds + abs-max reduce; the two
engines are balanced at ~14-17 us/tile.  Main loop ~93 us/core measured
on hardware via repeat-slope timing (vs ~125 us for the PE-transpose
variant, ~108 us for the fp32 baseline at 4x the bytes).
"""

import numpy as np

import jax
import jax.numpy as jnp
from jax.sharding import Mesh, PartitionSpec, NamedSharding

import concourse.bacc as bacc
import concourse.bass as bass
import concourse.mybir as mybir
import concourse.tile as tile
from concourse.bass2jax import _bass_exec_p, install_neuronx_cc_hook, partition_id_tensor
from concourse.masks import make_identity

try:
    from jax import shard_map as _shard_map_mod  # jax >= 0.8 path

    def shard_map(f, mesh, in_specs, out_specs, check_rep=False):
        return jax.shard_map(f, mesh=mesh, in_specs=in_specs, out_specs=out_specs,
                             check_vma=check_rep)
except Exception:  # pragma: no cover
    from jax.experimental.shard_map import shard_map as _shard_map_fn

    def shard_map(f, mesh, in_specs, out_specs, check_rep=False):
        return _shard_map_fn(f, mesh=mesh, in_specs=in_specs,
                             out_specs=out_specs, check_rep=check_rep)

N = 2048          # model dim (N_IN == N_OUT)
R = 64            # rank of U, V
R2 = 2 * R        # 128
NCORES = 8
TOK = 2048        # tokens per core (one batch element)
F32 = mybir.dt.float32
F16 = mybir.dt.float16
I8 = mybir.dt.int8
NCHUNK = N // 128          # 16 feature chunks
NTILE = TOK // 512         # 4 token tiles of 512
NSUB = 4                   # rows per partition within a tile
NBLK = N // 512            # 4 output feature blocks
QCAP = 126.0               # int8 target amplitude (margin below 127)

_NC_CACHE = {}
_FN_CACHE = {}


# --------------------------------------------------------------------------
# device kernel
# --------------------------------------------------------------------------

def _setup(nc, tc, ctx, uv_d, const):
    """Weight construction; returns (identH, Db_sb, S_sb) persistent tiles."""
    ident = const.tile([128, 128], F32)
    make_identity(nc, ident[:])
    identH = const.tile([128, 128], F16)
    nc.vector.tensor_copy(out=identH[:], in_=ident[:])
    Db_sb = const.tile([128, NCHUNK, 128], F16)
    S_sb = const.tile([128, N], F16)

    with tc.tile_pool(name="setup", bufs=1) as setup, \
         tc.tile_pool(name="ps_s", bufs=2, space="PSUM") as ps_s:
        # compact load: partition p holds C rows 16p..16p+15 (one 4KB desc each)
        Ctmp = setup.tile([128, NCHUNK, 128], F16)
        nc.sync.dma_start(out=Ctmp[:], in_=uv_d[:].rearrange("(p a) q -> p a q", p=128))

        # CT[rr, n] = C[n, rr] via 16 PE transposes (n = 16p + a), f32 upcast
        CT = setup.tile([128, N], F32)
        CTv = CT[:].rearrange("q (p a) -> q p a", a=NCHUNK)
        for a in range(NCHUNK):
            ps = ps_s.tile([128, 1024], F16, tag="tr16")
            nc.tensor.transpose(ps[:, 0:128], Ctmp[:, a, :], identH[:])
            nc.vector.tensor_copy(out=CTv[:, :, a], in_=ps[:, 0:128])

        # C_sb[p, j, rr] = C[128j + p, rr] via 16 PE transposes of CT blocks
        C_sb = setup.tile([128, NCHUNK, 128], F32)
        for j in range(NCHUNK):
            ps = ps_s.tile([128, 512], F32, tag="small_mm")
            nc.tensor.transpose(ps[:, 0:128], CT[:, j * 128 : (j + 1) * 128], ident[:])
            nc.vector.tensor_copy(out=C_sb[:, j, :], in_=ps[:, 0:128])

        # D = [V, -U]: swizzle halves of C
        D_sb = setup.tile([128, NCHUNK, 128], F32)
        nc.vector.tensor_copy(out=D_sb[:, :, 0:R], in_=C_sb[:, :, R:R2])
        nc.scalar.mul(D_sb[:, :, R:R2], C_sb[:, :, 0:R], -1.0)
        nc.vector.tensor_copy(out=Db_sb[:], in_=D_sb[:])

        counter = [0]

        def fresh():
            counter[0] += 1
            return setup.tile([128, 128], F32, name=f"sm{counter[0]}", tag=f"sm{counter[0]}")

        def accum_mm(lhs_view, rhs_view):
            ps = ps_s.tile([128, 512], F32, tag="small_mm")
            for j in range(NCHUNK):
                nc.tensor.matmul(
                    ps[:, 0:128],
                    lhs_view[:, j, :],
                    rhs_view[:, j, :],
                    start=(j == 0),
                    stop=(j == NCHUNK - 1),
                )
            out = fresh()
            nc.vector.tensor_copy(out=out[:], in_=ps[:, 0:128])
            return out

        def mm(lhsT, rhs):
            ps = ps_s.tile([128, 512], F32, tag="small_mm")
            nc.tensor.matmul(ps[:, 0:128], lhsT[:], rhs[:], start=True, stop=True)
            out = fresh()
            nc.vector.tensor_copy(out=out[:], in_=ps[:, 0:128])
            return out

        def add_i(a):
            out = fresh()
            nc.vector.tensor_add(out=out[:], in0=ident[:], in1=a[:])
            return out

        E = accum_mm(D_sb, C_sb)       # E = D^T C
        ET = accum_mm(C_sb, D_sb)      # E^T = C^T D
        E2 = mm(ET, E)
        E2T = mm(E, ET)
        E3 = mm(E2T, E)
        E4 = mm(E2T, E2)
        E4T = mm(E2, E2T)
        E7 = mm(E4T, E3)
        E8 = mm(E4T, E4)
        E8T = mm(E4, E4T)
        E15 = mm(E8T, E7)
        A1T = add_i(ET)
        A2 = add_i(E2)
        A4 = add_i(E4)
        A8 = add_i(E8)
        T1T = mm(A2, A1T)
        T2T = mm(A4, T1T)
        S16 = mm(T2T, A8)              # sum_{j=0}^{15} E^j
        F_sb = fresh()
        tmp2 = fresh()
        nc.vector.tensor_add(out=tmp2[:], in0=S16[:], in1=S16[:])
        nc.vector.tensor_sub(out=F_sb[:], in0=tmp2[:], in1=E15[:])

        # S = F^T C^T (fp32 matmul, rounded to f16 on copy-out)
        for nblk in range(NBLK):
            ps = ps_s.tile([128, 512], F32, tag="small_mm")
            nc.tensor.matmul(
                ps[:], F_sb[:], CT[:, nblk * 512 : (nblk + 1) * 512],
                start=True, stop=True,
            )
            nc.scalar.copy(out=S_sb[:, nblk * 512 : (nblk + 1) * 512], in_=ps[:])

    return identH, Db_sb, S_sb


def _main_loop(nc, tc, xb_d, yb_d, Db_sb, S_sb, scales, xs_sb, pools):
    xpool8, xtpool, ptpool, ypool, yqpool, mpool, ps_p, ps_c = pools
    # blocked layout: row t*128+p holds x^T[128j+p, 512t+s] at col j*512+s
    xb_r = xb_d[:].rearrange("(t p) f -> t p f", p=128)
    yb_r = yb_d[:].rearrange("(t p) f -> t p f", p=128)

    xb_tiles = {}
    xt_tiles = {}
    pt_tiles = {}

    def load(t):
        xb_t = xpool8.tile([128, NCHUNK, 512], I8, tag="xb_t", name=f"xb_t{t}")
        xb_tiles[t] = xb_t
        nc.sync.dma_start(out=xb_t[:].rearrange("p j s -> p (j s)"), in_=xb_r[t])

    def head(t):
        """dequant tile t into xt (f16), then stage1 -> pt."""
        xb_t = xb_tiles[t]
        xt = xtpool.tile([128, NCHUNK, 512], F16, tag="xt", name=f"xt{t}")
        xt_tiles[t] = xt
        for j in range(NCHUNK):
            sc = xs_sb[:, t * NCHUNK + j : t * NCHUNK + j + 1]
            nc.scalar.activation(
                out=xt[:, j, :], in_=xb_t[:, j, :],
                func=mybir.ActivationFunctionType.Copy, scale=sc,
            )
        psp = ps_p.tile([128, 512], F32, tag="ps_p")
        for j in range(NCHUNK):
            nc.tensor.matmul(
                psp[:],
                Db_sb[:, j, :],
                xt[:, j, :],
                start=(j == 0),
                stop=(j == NCHUNK - 1),
            )
        pt = ptpool.tile([128, 512], F16, tag="pt")
        nc.scalar.copy(out=pt[:], in_=psp[:])
        pt_tiles[t] = pt

    def tail(t):
        """stage2' corr^T = S^T P^T + residual + int8 quant + store."""
        xt = xt_tiles[t]
        pt = pt_tiles[t]
        y_t = ypool.tile([128, NCHUNK, 512], F16, tag="y_t")
        for j in range(NCHUNK):
            psc = ps_c.tile([128, 512], F32, tag="ps_c")
            nc.tensor.matmul(
                psc[:],
                S_sb[:, j * 128 : (j + 1) * 128],
                pt[:],
                start=True,
                stop=True,
            )
            nc.vector.tensor_add(
                out=y_t[:, j, :], in0=psc[:], in1=xt[:, j, :],
            )
        y_flat = y_t[:].rearrange("p j s -> p (j s)")
        m = mpool.tile([128, 1], F32, tag="m")
        nc.vector.tensor_reduce(
            out=m[:], in_=y_flat, axis=mybir.AxisListType.X,
            op=mybir.AluOpType.max, apply_absolute_value=True,
        )
        nc.vector.tensor_scalar(
            out=scales[:, t : t + 1], in0=m[:],
            scalar1=1.0 / QCAP, scalar2=1e-30,
            op0=mybir.AluOpType.mult, op1=mybir.AluOpType.max,
        )
        inv = mpool.tile([128, 1], F32, tag="inv")
        nc.vector.reciprocal(out=inv[:], in_=scales[:, t : t + 1])
        y_q = yqpool.tile([128, NCHUNK, 512], I8, tag="y_q")
        yq_flat = y_q[:].rearrange("p j s -> p (j s)")
        nc.scalar.activation(
            out=yq_flat[:], in_=y_flat[:],
            func=mybir.ActivationFunctionType.Copy, scale=inv[:],
        )
        nc.gpsimd.dma_start(out=yb_r[t], in_=y_q[:].rearrange("p j s -> p (j s)"))

    load(0)
    if NTILE > 1:
        load(1)
    for t in range(NTILE):
        if t + 2 < NTILE:
            load(t + 2)
        if t >= 1:
            tail(t - 1)
        head(t)
    tail(NTILE - 1)


def _emit(nc, tc, ctx, repeat=1):
    xb_d = nc.dram_tensor("xb", [NTILE * 128, NCHUNK * 512], I8, kind="ExternalInput")
    xs_d = nc.dram_tensor("xs", [128, NTILE * NCHUNK], F32, kind="ExternalInput")
    uv_d = nc.dram_tensor("uv", [N, R2], F16, kind="ExternalInput")
    yb_d = nc.dram_tensor("yb", [NTILE * 128, NCHUNK * 512], I8, kind="ExternalOutput")
    ys_d = nc.dram_tensor("ys", [128, NTILE], F32, kind="ExternalOutput")

    const = ctx.enter_context(tc.tile_pool(name="const", bufs=1))

    identH, Db_sb, S_sb = _setup(nc, tc, ctx, uv_d, const)

    scales = const.tile([128, NTILE], F32)
    xs_sb = const.tile([128, NTILE * NCHUNK], F32)
    nc.sync.dma_start(out=xs_sb[:], in_=xs_d[:])

    xpool8 = ctx.enter_context(tc.tile_pool(name="xpool8", bufs=3))
    xtpool = ctx.enter_context(tc.tile_pool(name="xtpool", bufs=2))
    ptpool = ctx.enter_context(tc.tile_pool(name="ptpool", bufs=2))
    ypool = ctx.enter_context(tc.tile_pool(name="ypool", bufs=2))
    yqpool = ctx.enter_context(tc.tile_pool(name="yqpool", bufs=2))
    mpool = ctx.enter_context(tc.tile_pool(name="mpool", bufs=2))
    ps_p = ctx.enter_context(tc.tile_pool(name="ps_p", bufs=2, space="PSUM"))
    ps_c = ctx.enter_context(tc.tile_pool(name="ps_c", bufs=4, space="PSUM"))
    pools = (xpool8, xtpool, ptpool, ypool, yqpool, mpool, ps_p, ps_c)

    def main_body():
        _main_loop(nc, tc, xb_d, yb_d, Db_sb, S_sb, scales, xs_sb, pools)
        nc.sync.dma_start(out=ys_d[:], in_=scales[:])

    if repeat > 1:
        with tc.For_i(0, repeat, 1):
            main_body()
    else:
        main_body()


def build_nc(repeat=1, **_ignored):
    key = ("v10", repeat)
    if key in _NC_CACHE:
        return _NC_CACHE[key]
    nc = bacc.Bacc(
        "TRN2",
        target_bir_lowering=False,
        debug=False,
        enable_asserts=False,
        num_devices=NCORES,
    )
    from contextlib import ExitStack

    with tile.TileContext(nc) as tc, ExitStack() as ctx:
        _emit(nc, tc, ctx, repeat=repeat)
    nc.compile()
    _NC_CACHE[key] = nc
    return nc


# --------------------------------------------------------------------------
# host run path (custom PJRT invocation: no host-side zero upload)
# --------------------------------------------------------------------------

def _get_fn(repeat=1):
    key = ("fn", repeat)
    if key in _FN_CACHE:
        return _FN_CACHE[key]
    nc = build_nc(repeat=repeat)
    install_neuronx_cc_hook()

    partition_name = nc.partition_id_tensor.name if nc.partition_id_tensor else None
    in_names, out_names, out_avals = [], [], []
    for alloc in nc.m.functions[0].allocations:
        if not isinstance(alloc, mybir.MemoryLocationSet):
            continue
        name = alloc.memorylocations[0].name
        if alloc.kind == "ExternalInput":
            if name != partition_name:
                in_names.append(name)
        elif alloc.kind == "ExternalOutput":
            out_names.append(name)
            out_avals.append(
                jax.core.ShapedArray(tuple(alloc.tensor_shape), mybir.dt.np(alloc.dtype))
            )
    n_params = len(in_names)
    n_outs = len(out_names)
    all_in_names = tuple(in_names) + tuple(out_names)
    if partition_name is not None:
        all_in_names = all_in_names + (partition_name,)

    def _body(*args):
        operands = list(args)
        if partition_name is not None:
            operands.append(partition_id_tensor())
        outs = _bass_exec_p.bind(
            *operands,
            out_avals=tuple(out_avals),
            in_names=all_in_names,
            out_names=tuple(out_names),
            lowering_input_output_aliases=(),
            sim_require_finite=False,
            sim_require_nnan=False,
            nc=nc,
        )
        return tuple(outs)

    devices = jax.devices()[:NCORES]
    mesh = Mesh(np.asarray(devices), ("core",))
    spec = PartitionSpec("core")
    in_specs = (spec,) * (n_params + n_outs)
    out_specs = (spec,) * n_outs
    donate = tuple(range(n_params, n_params + n_outs))
    fn = jax.jit(
        shard_map(_body, mesh=mesh, in_specs=in_specs, out_specs=out_specs,
                  check_rep=False),
        donate_argnums=donate,
        keep_unused=True,
    )
    sharding = NamedSharding(mesh, spec)

    def _make_outbufs():
        return tuple(
            jnp.zeros((NCORES * av.shape[0], *av.shape[1:]), av.dtype)
            for av in out_avals
        )

    make_outbufs_sharded = jax.jit(
        _make_outbufs,
        out_shardings=tuple(sharding for _ in range(n_outs)),
    )
    info = (fn, tuple(in_names), tuple(out_names), make_outbufs_sharded, sharding)
    _FN_CACHE[key] = info
    return info


def _run(input, U, V, repeat=1, **_ignored):
    fn, in_names, out_names, make_outbufs, sharding = _get_fn(repeat=repeat)

    uv = np.concatenate([U, V], axis=1).astype(np.float16)
    xf = np.asarray(input, dtype=np.float32)
    # blocked transpose: xb[c, t, p, j, s] = x[c, 512t+s, 128j+p]
    xr = xf.reshape(NCORES, NTILE, 512, NCHUNK, 128).transpose(0, 1, 4, 3, 2)
    mx = np.abs(xr).max(axis=4, keepdims=True)               # per (c,t,p,j)
    sx = (mx / 127.0).astype(np.float32)
    np.maximum(sx, 1e-30, out=sx)
    xb = np.rint(xr / sx).astype(np.int8).reshape(NCORES * NTILE * 128, NCHUNK * 512)
    # xs layout: [c, p, t*NCHUNK + j]
    xs = np.ascontiguousarray(
        sx[:, :, :, :, 0].transpose(0, 2, 1, 3).reshape(NCORES, 128, NTILE * NCHUNK)
    ).reshape(NCORES * 128, NTILE * NCHUNK)
    host_in = {"xb": xb, "xs": xs, "uv": np.concatenate([uv] * NCORES, axis=0)}
    args = [host_in[name] for name in in_names]
    outbufs = make_outbufs()
    res = fn(*args, *outbufs)
    res = {name: np.asarray(r) for name, r in zip(out_names, res)}

    yb = res["yb"].reshape(NCORES, NTILE, 128, NCHUNK, 512)
    ys = res["ys"].reshape(NCORES, 128, NTILE)
    scl = ys.transpose(0, 2, 1)[:, :, :, None, None]          # [c, t, p, 1, 1]
    out = np.ascontiguousarray(
        (yb.astype(np.float32) * scl).transpose(0, 1, 4, 3, 2)
    ).reshape(NCORES, TOK, N)
    return out, res


def kernel(input, U, V):
    out, _ = _run(input, U, V)
    return out
